# revision 5
# baseline (speedup 1.0000x reference)
"""Trainium2 Bass kernel for nn_AttentiveEncoderPOS — fp8 DR + AG-overlapped.

vs kernel_v4:
  - ONE combined AllGather (fp8, 2MB/rank) instead of two (two collectives
    paid two ncfw floors and serialized; one is strictly faster).
  - Phase 2 processes the core's OWN key block first, entirely from SBUF
    (qpack/vpack), overlapping the AllGather; the 7 remote blocks are
    fetched with pid-dependent indirect gathers (rank = (pid+b) % 8), one
    [128x8]-offset indirect DMA per rank per tensor, landing in the same
    [p, (tile, col)] layout as the local packs.
  - A@V(b) emitted right after scores(b); colsum spans all 64 tiles.
"""

import numpy as np

import concourse.bass as bass
import concourse.mybir as mybir
from concourse import bacc
from concourse.tile import TileContext
from concourse.bass_utils import run_bass_kernel_spmd
from concourse.masks import make_identity

N = 8192
H = 1024
VOCAB = 50257
POS = 64
NCORES = 8
NL = N // NCORES          # 1024 rows (queries) per core
P = 128
HT = H // P               # 8 h tiles
K2 = 2 * H
KTI = K2 // P             # 16 contraction tiles for the linear
RT = NL // P              # 8 row tiles per core
KT = N // P               # 64 key tiles globally
QTN = NL // P             # 8 query tiles
BLK = 8                   # key tiles per phase-2 block (= one rank)
ESC = 64.0                # emb scale
WSC = 16.0                # W scale
LSCALE = ESC * WSC        # L' = 1024 * L
SCALE = 1.0 / (32.0 * LSCALE * LSCALE)

BF = mybir.dt.bfloat16
F8 = mybir.dt.float8e4
F32 = mybir.dt.float32
I32 = mybir.dt.int32
U32 = mybir.dt.uint32
EXP = mybir.ActivationFunctionType.Exp
DR = mybir.MatmulPerfMode.DoubleRow
ADD = mybir.AluOpType.add
MULT = mybir.AluOpType.mult
IS_GE = mybir.AluOpType.is_ge


def build_nc():
    nc = bacc.Bacc(num_devices=NCORES)
    ids = nc.declare_dram_parameter("ids", [RT, P, 1], I32, isOutput=False)
    pids = nc.declare_dram_parameter("pids", [RT, P, 1], I32, isOutput=False)
    emb = nc.declare_dram_parameter("emb", [VOCAB, H], BF, isOutput=False)
    pemb = nc.declare_dram_parameter("pemb", [POS, H], BF, isOutput=False)
    wt = nc.declare_dram_parameter("wt", [KTI, P, H], F8, isOutput=False)
    bias = nc.declare_dram_parameter("bias", [HT, P, 1], F32, isOutput=False)
    out = nc.declare_dram_parameter("out", [NL, H], F32, isOutput=True)

    # split AllGather bounces, per-partition-major: row p holds that lane's
    # L'.T (ht, c) block / V' (t, h) block, 8KB contiguous each.
    HW_ = HT * NL  # 8192
    cc_lt_in = nc.dram_tensor("cc_lt_in", [P, HW_], F8)
    cc_lt_out = nc.dram_tensor("cc_lt_out", [NCORES, P, HW_], F8, addr_space="Shared")
    cc_lt_flat = cc_lt_out.rearrange("r p c -> (r p) c")  # [1024, 8192]
    cc_v_in = nc.dram_tensor("cc_v_in", [P, HW_], F8)
    cc_v_out = nc.dram_tensor("cc_v_out", [NCORES, P, HW_], F8, addr_space="Shared")
    cc_v_flat = cc_v_out.rearrange("r p c -> (r p) c")
    cs_d = nc.dram_tensor("cs_d", [NL], F32)  # colsum row->col bounce

    with TileContext(nc) as tc:
        with (
            tc.tile_pool(name="const", bufs=1) as const,
            tc.tile_pool(name="ltq", bufs=1) as ltq,
            tc.tile_pool(name="vpk", bufs=1) as vpk,
            tc.tile_pool(name="idxp", bufs=NCORES) as idxp,
        ):
            ident = const.tile([P, P], BF)
            make_identity(nc, ident[:])
            ones2 = const.tile([P, 32], F8)  # k-stride 16B for DR lhsT
            nc.gpsimd.memset(ones2[:], 1.0)
            b_sb = const.tile([P, HT], F32)
            nc.sync.dma_start(
                out=b_sb[:].rearrange("p (h u) -> p h u", h=HT),
                in_=bias.rearrange("h p u -> p h u"),
            )

            # Q.T / own keys and own V, fp8, packed [128, (tile col)]
            qpack = ltq.tile([P, HT * NL], F8, tag="qp", name="qpack")
            qp3 = qpack[:].rearrange("p (h q) -> p h q", h=HT)
            vpack = vpk.tile([P, RT * H], F8, tag="vp", name="vpack")
            vp3 = vpack[:].rearrange("p (t h) -> p t h", t=RT)

            # ---- gather row-indices for the 7 remote ranks ----
            # idx[b][p] = ((pid+1+b)%8)*128 + p
            iota1 = idxp.tile([P, 1], I32, tag="iota1")
            nc.gpsimd.iota(iota1[:], pattern=[[0, 1]], base=0, channel_multiplier=1)
            iota1f = idxp.tile([P, 1], F32, tag="iota1f")
            nc.vector.tensor_copy(out=iota1f[:], in_=iota1[:])
            pid_u = idxp.tile([P, 1], U32, tag="pidu")
            nc.sync.dma_start(
                out=pid_u[:], in_=nc.partition_id_tensor.ap().to_broadcast([P, 1])
            )
            pid_f = idxp.tile([P, 1], F32, tag="pidf")
            nc.vector.tensor_copy(out=pid_f[:], in_=pid_u[:])
            idx_g = []
            for b in range(1, NCORES):
                rb = idxp.tile([P, 1], F32, tag="rb", name="rb")
                # rb = pid + b ; rb -= 8*(rb >= 8) ; rb = rb*128 + p
                nc.vector.tensor_scalar(
                    out=rb[:], in0=pid_f[:], scalar1=float(b), scalar2=None, op0=ADD
                )
                ge = idxp.tile([P, 1], F32, tag="ge", name="ge")
                nc.vector.tensor_scalar(
                    out=ge[:], in0=rb[:], scalar1=8.0, scalar2=-8.0 * 128.0,
                    op0=IS_GE, op1=MULT,
                )
                nc.vector.tensor_scalar(
                    out=rb[:], in0=rb[:], scalar1=128.0, scalar2=None, op0=MULT
                )
                nc.vector.tensor_tensor(out=rb[:], in0=rb[:], in1=ge[:], op=ADD)
                nc.vector.tensor_tensor(out=rb[:], in0=rb[:], in1=iota1f[:], op=ADD)
                il = idxp.tile([P, 1], I32, tag="il", name="il")
                nc.vector.tensor_copy(out=il[:], in_=rb[:])
                idx_g.append(il)

            # ---------------- Phase 1: local linear (fp8 DR) ----------------
            with (
                tc.tile_pool(name="wxp", bufs=2) as wxp,
                tc.tile_pool(name="idp", bufs=2) as idp,
                tc.tile_pool(name="xbp", bufs=RT) as xbp,
                tc.tile_pool(name="lbf", bufs=HT) as lbf,
                tc.tile_pool(name="tps", bufs=2, space="PSUM") as tps,
                tc.tile_pool(name="mps", bufs=2, space="PSUM") as mps,
            ):
                idt = idp.tile([P, RT], I32, tag="id")
                nc.sync.dma_start(
                    out=idt[:].rearrange("p (t u) -> p t u", t=RT),
                    in_=ids.rearrange("t p u -> p t u"),
                )
                pidt = idp.tile([P, RT], I32, tag="pid")
                nc.sync.dma_start(
                    out=pidt[:].rearrange("p (t u) -> p t u", t=RT),
                    in_=pids.rearrange("t p u -> p t u"),
                )
                xes = []
                xps = []
                for rt in range(RT):
                    xe = xbp.tile([P, H], BF, tag="xe", name="xe")
                    nc.gpsimd.indirect_dma_start(
                        out=xe[:],
                        out_offset=None,
                        in_=emb[:],
                        in_offset=bass.IndirectOffsetOnAxis(
                            ap=idt[:, rt : rt + 1], axis=0
                        ),
                    )
                    xes.append(xe)
                for rt in range(RT):
                    xp_t = xbp.tile([P, H], BF, tag="xpe", name="xp_t")
                    nc.gpsimd.indirect_dma_start(
                        out=xp_t[:],
                        out_offset=None,
                        in_=pemb[:],
                        in_offset=bass.IndirectOffsetOnAxis(
                            ap=pidt[:, rt : rt + 1], axis=0
                        ),
                    )
                    xps.append(xp_t)
                wpack = wxp.tile([P, KTI * H], F8, tag="wp", name="wpack")
                nc.sync.dma_start(
                    out=wpack[:].rearrange("p (k h) -> p k h", k=KTI),
                    in_=wt.rearrange("k p h -> p k h"),
                )
                wp3 = wpack[:].rearrange("p (k h) -> p k h", k=KTI)
                xpack = wxp.tile([P, KTI * NL], F8, tag="xp", name="xpack")
                xp3 = xpack[:].rearrange("p (k q) -> p k q", k=KTI)
                for k in range(KTI):
                    pt = tps.tile([P, NL], BF, tag="tp")
                    srcs = xes if k < HT else xps
                    kk2 = k if k < HT else k - HT
                    for rt in range(RT):
                        nc.tensor.transpose(
                            pt[:, rt * P : (rt + 1) * P],
                            srcs[rt][:, kk2 * P : (kk2 + 1) * P],
                            ident[:],
                        )
                    nc.vector.tensor_copy(
                        out=xpack[:, k * NL : (k + 1) * NL], in_=pt[:]
                    )
                lt_bf = []
                for ht in range(HT):
                    pm = mps.tile([P, NL], F32, tag="pm")
                    for half in range(2):
                        sl = slice(half * 512, (half + 1) * 512)
                        for kp in range(KTI // 2):
                            nc.tensor.matmul(
                                pm[:, sl],
                                lhsT=wp3[:, 2 * kp : 2 * kp + 2, ht * P : (ht + 1) * P],
                                rhs=xp3[:, 2 * kp : 2 * kp + 2, sl],
                                start=(kp == 0),
                                stop=(kp == KTI // 2 - 1),
                                perf_mode=DR,
                            )
                    nc.vector.tensor_add(
                        out=qpack[:, ht * NL : (ht + 1) * NL],
                        in0=pm[:],
                        in1=b_sb[:, ht : ht + 1].to_broadcast([P, NL]),
                    )
                    lb = lbf.tile([P, NL], BF, tag="lb", name="lb")
                    nc.vector.tensor_add(
                        out=lb[:],
                        in0=pm[:],
                        in1=b_sb[:, ht : ht + 1].to_broadcast([P, NL]),
                    )
                    lt_bf.append(lb)
                    nc.sync.dma_start(
                        out=cc_lt_in[:, ht * NL : (ht + 1) * NL],
                        in_=qpack[:, ht * NL : (ht + 1) * NL],
                    )
                nc.gpsimd.collective_compute(
                    "AllGather",
                    mybir.AluOpType.bypass,
                    replica_groups=[list(range(NCORES))],
                    ins=[cc_lt_in.ap().opt()],
                    outs=[cc_lt_out.ap().opt()],
                )
                for rt in range(RT):
                    pv = tps.tile([P, H], BF, tag="pv")
                    for ht in range(HT):
                        nc.tensor.transpose(
                            pv[:, ht * P : (ht + 1) * P],
                            lt_bf[ht][:, rt * P : (rt + 1) * P],
                            ident[:],
                        )
                    nc.vector.tensor_copy(
                        out=vpack[:, rt * H : (rt + 1) * H], in_=pv[:]
                    )
                    nc.sync.dma_start(
                        out=cc_v_in[:, rt * H : (rt + 1) * H],
                        in_=vpack[:, rt * H : (rt + 1) * H],
                    )

            nc.gpsimd.collective_compute(
                "AllGather",
                mybir.AluOpType.bypass,
                replica_groups=[list(range(NCORES))],
                ins=[cc_v_in.ap().opt()],
                outs=[cc_v_out.ap().opt()],
            )

            # ---------------- Phase 2: attention (fp8 DR) ----------------
            with (
                tc.tile_pool(name="ltg", bufs=4) as ltgp,
                tc.tile_pool(name="vg", bufs=4) as vgp,
                tc.tile_pool(name="ep", bufs=BLK + 2) as ep,
                tc.tile_pool(name="op", bufs=QTN) as op,
                tc.tile_pool(name="fin", bufs=2) as fin,
                tc.tile_pool(name="sps", bufs=2, space="PSUM") as sps,
                tc.tile_pool(name="ops", bufs=2, space="PSUM") as ops,
                tc.tile_pool(name="cps", bufs=1, space="PSUM") as cps,
            ):
                psum_c = [
                    cps.tile([1, 512], F32, tag="cs0", name="psum_c0"),
                    cps.tile([1, 512], F32, tag="cs1", name="psum_c1"),
                ]
                out_sb = [op.tile([P, H], F32, tag="o", name="o") for _ in range(QTN)]
                rec = None

                lt_srcs = [qp3]
                v_srcs = [vp3]

                def emit_gathers():
                    # remote gathers; lt ones only need the early lt AG
                    for b in range(1, NCORES):
                        g = ltgp.tile([P, HW_], F8, tag="ltg", name="g")
                        nc.gpsimd.indirect_dma_start(
                            out=g[:],
                            out_offset=None,
                            in_=cc_lt_flat[:],
                            in_offset=bass.IndirectOffsetOnAxis(
                                ap=idx_g[b - 1][:, 0:1], axis=0
                            ),
                        )
                        lt_srcs.append(g[:].rearrange("p (h q) -> p h q", h=HT))
                    for b in range(1, NCORES):
                        gv = vgp.tile([P, HW_], F8, tag="vg", name="gv")
                        nc.gpsimd.indirect_dma_start(
                            out=gv[:],
                            out_offset=None,
                            in_=cc_v_flat[:],
                            in_offset=bass.IndirectOffsetOnAxis(
                                ap=idx_g[b - 1][:, 0:1], axis=0
                            ),
                        )
                        v_srcs.append(gv[:].rearrange("p (t h) -> p t h", t=RT))

                e2s_all = []
                for b in range(NCORES + 1):
                    if b == 1:
                        emit_gathers()
                    lt3 = lt_srcs[b] if b < NCORES else None
                    e2s = []
                    # scores + exp for this block's 8 key tiles
                    for j in range(BLK if b < NCORES else 0):
                        kt = b * BLK + j
                        kk = j % 2
                        if kk == 0:
                            e2 = ep.tile([P, 2 * NL], F8, tag="e2", name="e2")
                            e2s.append(e2)
                        for qc in range(2):
                            sl = slice(qc * 512, (qc + 1) * 512)
                            ps = sps.tile([P, 512], F32, tag="sp")
                            for hp in range(HT // 2):
                                nc.tensor.matmul(
                                    ps[:],
                                    lhsT=lt3[:, 2 * hp : 2 * hp + 2, j * P : (j + 1) * P],
                                    rhs=qp3[:, 2 * hp : 2 * hp + 2, sl],
                                    start=(hp == 0),
                                    stop=(hp == HT // 2 - 1),
                                    perf_mode=DR,
                                )
                            nc.scalar.activation(
                                out=e2[:, kk * NL + qc * 512 : kk * NL + (qc + 1) * 512],
                                in_=ps[:],
                                func=EXP,
                                scale=SCALE,
                            )
                        if kk == 1:
                            e2v = e2[:].rearrange("p (k q) -> p k q", k=2)
                            o2 = ones2[:].rearrange("p (k u) -> p k u", k=2)[:, :, 0:1]
                            for qc in range(2):
                                sl = slice(qc * 512, (qc + 1) * 512)
                                nc.tensor.matmul(
                                    psum_c[qc][:, :],
                                    lhsT=o2,
                                    rhs=e2v[:, :, sl],
                                    start=(kt == 1),
                                    stop=(kt == KT - 1),
                                    perf_mode=DR,
                                )
                    e2s_all.append(e2s)
                    if b == NCORES - 1:
                        # normalization factors: runs while A@V(last) computes
                        cs_row = fin.tile([1, NL], F32, tag="csr")
                        nc.vector.tensor_copy(out=cs_row[0:1, 0:512], in_=psum_c[0][:])
                        nc.vector.tensor_copy(out=cs_row[0:1, 512:NL], in_=psum_c[1][:])
                        nc.sync.dma_start(out=cs_d[:], in_=cs_row[0:1, :])
                        cs_cols = fin.tile([P, QTN], F32, tag="csc")
                        nc.sync.dma_start(
                            out=cs_cols[:], in_=cs_d.rearrange("(q p) -> p q", p=P)
                        )
                        cs_sc = fin.tile([P, QTN], F32, tag="cssc")
                        nc.vector.tensor_scalar_mul(
                            out=cs_sc[:], in0=cs_cols[:], scalar1=LSCALE
                        )
                        rec = fin.tile([P, QTN], F32, tag="rec")
                        nc.vector.reciprocal(rec[:], cs_sc[:])
                    # A@V for the PREVIOUS block (skewed pipeline)
                    if b == 0:
                        continue
                    ab = b - 1
                    v3 = v_srcs[ab]
                    e2p = e2s_all[ab]
                    last = b == NCORES
                    for qt in range(QTN):
                        po = ops.tile([P, H], F32, tag="po")
                        for pr in range(BLK // 2):
                            e2v = e2p[pr][:].rearrange("p (k q) -> p k q", k=2)
                            for hh in range(2):
                                sl = slice(hh * 512, (hh + 1) * 512)
                                nc.tensor.matmul(
                                    po[:, sl],
                                    lhsT=e2v[:, :, qt * P : (qt + 1) * P],
                                    rhs=v3[:, 2 * pr : 2 * pr + 2, sl],
                                    start=(pr == 0),
                                    stop=(pr == BLK // 2 - 1),
                                    perf_mode=DR,
                                )
                        if ab == 0:
                            nc.vector.tensor_copy(out=out_sb[qt][:], in_=po[:])
                        else:
                            nc.vector.tensor_add(
                                out=out_sb[qt][:], in0=out_sb[qt][:], in1=po[:]
                            )
                        if last:
                            nc.vector.tensor_mul(
                                out=out_sb[qt][:],
                                in0=out_sb[qt][:],
                                in1=rec[:, qt : qt + 1].to_broadcast([P, H]),
                            )
                            nc.sync.dma_start(
                                out=out[qt * P : (qt + 1) * P, :], in_=out_sb[qt][:]
                            )
    nc.finalize()
    return nc


def _prep_inputs(inputs):
    bf = mybir.dt.np(BF)
    f8 = mybir.dt.np(F8)
    ids = np.asarray(inputs["input_ids"]).astype(np.int32)
    pids = np.asarray(inputs["pos_ids"]).astype(np.int32)
    emb = (np.asarray(inputs["emb"], dtype=np.float32) * ESC).astype(bf)
    pemb = (np.asarray(inputs["pos_emb"], dtype=np.float32) * ESC).astype(bf)
    W = np.asarray(inputs["W"], dtype=np.float32)
    wt = np.ascontiguousarray(W.T * WSC).astype(f8).reshape(KTI, P, H)
    b = np.asarray(inputs["b"], dtype=np.float32) * LSCALE
    bias = np.ascontiguousarray(b.reshape(HT, P, 1))
    in_maps = []
    for i in range(NCORES):
        in_maps.append(
            {
                "ids": np.ascontiguousarray(ids[i * NL : (i + 1) * NL].reshape(RT, P, 1)),
                "pids": np.ascontiguousarray(
                    pids[i * NL : (i + 1) * NL].reshape(RT, P, 1)
                ),
                "emb": emb,
                "pemb": pemb,
                "wt": wt,
                "bias": bias,
            }
        )
    return in_maps


def run(inputs, trace=False):
    nc = build_nc()
    in_maps = _prep_inputs(inputs)
    res = run_bass_kernel_spmd(nc, in_maps, list(range(NCORES)), trace=trace)
    out = np.concatenate([res.results[i]["out"] for i in range(NCORES)], axis=0)
    return out, res


def kernel(**inputs):
    out, _ = run(inputs, trace=False)
    return out
